# revision 4
# baseline (speedup 1.0000x reference)
"""CFConvCluster Trainium2 kernel (8 NeuronCores, SPMD, no collectives).

Strategy
--------
The reference computes, per edge e:  msg_e = mask_e * new_node[src_e] * MLP(rbf_e)
and scatter-sums msg into dst nodes.  Exact algebraic restructurings:

1. Masked edges contribute exactly zero -> dropped up front (E: 1.6M -> ~449k).
2. Nodes are relabeled (host permutation) into 896 in-degree-balanced
   112-node "windows"; edges grouped by the window of their dst.  The
   segment-sum for a window is a matmul with a one-hot selection matrix
   S_T[e, n] = (dst_e == slot n), accumulated over the window's T edge
   tiles in PSUM.  Output ranges are disjoint across cores -> no
   all-reduce; the host concatenates and un-permutes rows.
3. b2 folds via linearity: sum S*(h2+b2)*g = sum S*(h2*g) + b2*sum S*g,
   twin accumulated matmuls sharing one PSUM tile (skipped entirely when
   b2 == 0, as in the reference data).

v2 core scheme (kept):
- rbf + W1 are fp8(e4m3); MM1 is ONE block-diagonal DoubleRow matmul per
  pair (2 fp8/cell => K=128 packed as 64 partitions x 2; the
  [[W1,0],[0,W1]] stationary routes window A to out partitions 0:64 and
  B to 64:128 in a single pass over the full virtual 128x256 array).
- MM2 is pair-merged: one [128,128] block-diag weight ([[2W2,0],[0,2W2]])
  turns h1[:, t-tile] directly into (h2_A | h2_B) per edge column, so
  the gather-multiply is a single DVE op per pair over one PSUM bank.
- 3-stage software pipeline over superpairs (2 pairs/iteration):
  MM1+softplus+S_T builds for i; MM2+multiply for i-1; scatter+staging+
  store for i-2.
- The softplus is the proven Exp+Ln two-pass on ScalarE (single-table
  'natural_log_exp_and_others'); the hardware Softplus LUT mis-evaluates.

v3 (this file): DMA instruction diet + SWDGE dispatch.
- The v2 HW/model gap traced to DMA dispatch: every HWDGE DMA occupies
  the issuing queue (SP) ~1.3us and the shared HWDGE descriptor
  generator ~625ns.  62 DMAs/core => the SP queue was ~94% busy just
  issuing DMAs, and 8 cores contend for descriptor-gen.
- All DRAM traffic now moves in ONE DMA per 4-pair group, per
  direction: rbf8+gath bytes are host-packed into a single fp8-
  container row (the bf16 gather half is viewed via AP.bitcast on
  device), and the output staging tile is flushed once per group.  All
  six persistent tensors are packed into one [128, 5000B] row.
- Every DMA is issued from the GpSimd queue (SWDGE): dispatch costs
  ~25ns of queue time (vs 1.2us+ on SP/ACT HWDGE rings) and descriptor
  generation runs on the otherwise-idle GpSimd engine (~1us/DMA),
  off the shared HWDGE unit entirely.  Total: 62 -> ~31 DMAs/core.
"""

import os
import numpy as np

N_NODES = 100_000
RBF = 128
DIM = 64
CORES = 8
WSZ = 112                  # nodes per window (PSUM out cols; <= 128)
W_TOTAL = 896              # 896*112 = 100352 >= N_NODES; divisible by 8
WPC = W_TOTAL // CORES     # 112 windows per core
NPAIR = WPC // 2
GRP = 4                    # pairs per DMA group (one DMA instr per group)
NODES_CAP = W_TOTAL * WSZ  # 100352


# ----------------------------------------------------------------------------
# Host-side preprocessing
# ----------------------------------------------------------------------------

def _prepare(rbf, new_node, src, dst, edge_mask, W1, b1, W2, b2):
    import ml_dtypes
    bf = ml_dtypes.bfloat16
    f8 = ml_dtypes.float8_e4m3fn
    f32 = np.float32

    mask = np.asarray(edge_mask).astype(bool)
    kept = np.nonzero(mask)[0]
    src_k = np.asarray(src)[kept].astype(np.int64)
    dst_k = np.asarray(dst)[kept].astype(np.int64)
    Ek = len(kept)

    # --- node -> (window, slot) assignment, balanced by in-degree ---
    deg = np.bincount(dst_k, minlength=NODES_CAP)
    order = np.argsort(-deg, kind="stable")
    node_win = np.empty(NODES_CAP, np.int64)
    node_slot = np.empty(NODES_CAP, np.int64)
    o2 = order.reshape(WSZ, W_TOTAL)
    wins = np.tile(np.arange(W_TOTAL), (WSZ, 1))
    wins[1::2] = wins[1::2, ::-1]          # serpentine deal
    node_win[o2] = wins
    node_slot[o2] = np.arange(WSZ)[:, None]

    ewin = node_win[dst_k]
    loads = np.bincount(ewin, minlength=W_TOTAL)
    T = max(2, int(np.ceil(loads.max() / 128)))  # tiles of 128 edges / window
    EW = T * 128
    EPAD = W_TOTAL * EW

    # --- edge placement: group edges by window, pad windows to EW ---
    order_e = np.argsort(ewin, kind="stable")
    ewin_s = ewin[order_e]
    cum = np.concatenate([[0], np.cumsum(loads)])
    pos = (np.arange(Ek) - cum[ewin_s]) + ewin_s * EW  # padded slot per edge

    dstoff_full = np.zeros(EPAD, f32)
    dstoff_full[pos] = node_slot[dst_k[order_e]]

    rbf_full = np.zeros((EPAD, RBF), f8)
    rbf_full[pos] = np.asarray(rbf, f32)[kept[order_e]].astype(f8)

    # Host-staged gather of source-node features into padded edge order.
    # (Padding/masked slots stay zero, which also implements edge masking.)
    gath_full = np.zeros((EPAD, DIM), bf)
    gath_full[pos] = np.asarray(new_node, f32)[src_k[order_e]].astype(bf)

    # --- per-core tensors, ONE fully-contiguous DRAM row per GRP pairs ---
    # rbf8 group row (g, p) cols (j, s, e): pair q = g*GRP+j, window A=2q on
    # partitions 0:64, B on 64:128; feature f = s*64 + p (DoubleRow packing).
    NG = NPAIR // GRP
    a = rbf_full.reshape(CORES, NG, GRP, 2, EW, 2, 64)     # [c,g,j,ab,e,s,p]
    rbf8_c = np.ascontiguousarray(
        a.transpose(0, 1, 3, 6, 2, 5, 4)                   # [c,g,ab,p,j,s,e]
        .reshape(CORES, NG * 128, GRP * 2 * EW))

    # gath group row (g, p=edge-in-tile) cols (j, t, ab, d) to match the
    # pair-merged MM2 output column order.
    g = gath_full.reshape(CORES, NG, GRP, 2, T, 128, 64)   # [c,g,j,ab,t,p,d]
    gath_c = np.ascontiguousarray(
        g.transpose(0, 1, 5, 2, 4, 3, 6)                   # [c,g,p,j,t,ab,d]
        .reshape(CORES, NG * 128, GRP * T * 128))

    # Single fp8-container pack per group row: rbf8 bytes | gath bytes.
    pack_c = np.ascontiguousarray(np.concatenate(
        [rbf8_c.view(np.uint8), gath_c.view(np.uint8)], axis=2)).view(f8)

    d0 = dstoff_full.reshape(CORES, WPC, T, 128)
    dstof_c = np.ascontiguousarray(
        d0.transpose(0, 3, 1, 2).reshape(CORES, 128, WPC * T).astype(bf))

    # iota_rep[p, n*4T + j] = n (fully packed operand for the superpair S_T
    # build: one EQ op covers all 4 windows, j = (2a+sub)*T + t)
    iota = np.ascontiguousarray(np.repeat(
        np.arange(WSZ, dtype=f32), 4 * T)[None, :].repeat(128, 0).astype(bf))

    # W1 fp8 DoubleRow operand, BLOCK-DIAGONAL so one matmul computes both
    # windows of a pair (A on partitions/out-dims 0:64, B on 64:128) with
    # the output based at partition 0 (dst offset 64 is invalid ISA for
    # DoubleRow): w1blk[p, s, m] = W1[s*64 + (p%64), m%64] on the diagonal
    # blocks (p<64 & m<64, p>=64 & m>=64), zero elsewhere.
    w1q = np.asarray(W1, f32).astype(f8)                   # [128, 64]
    wa = w1q.reshape(2, 64, 64).transpose(1, 0, 2)         # [p64, s, m]
    w1blk = np.zeros((128, 2, 128), f8)
    w1blk[0:64, :, 0:64] = wa
    w1blk[64:128, :, 64:128] = wa
    w1dr = np.ascontiguousarray(w1blk.reshape(128, 256))

    w2blk = np.zeros((128, 128), f32)
    w2blk[0:64, 0:64] = 2.0 * np.asarray(W2, f32)
    w2blk[64:128, 64:128] = 2.0 * np.asarray(W2, f32)
    w2blk = np.ascontiguousarray(w2blk.astype(bf))

    b1h2 = np.ascontiguousarray(np.tile(
        0.5 * np.asarray(b1, f32)[:, None], (2, 1)))       # [128, 1]
    b2c = np.zeros((128, 1), f32)
    b2c[0:64] = np.asarray(b2, f32)[:, None]               # rows 64: pad

    # --- persistent pack: one [128, 5000B] row, bitcast-viewed on device ---
    # layout (bytes): w1dr 256 | w2blk 256 | iota 3584 | dstof 896 | b1h2 4
    #                 | b2c 4
    in_maps = []
    for c in range(CORES):
        per_c = np.concatenate(
            [w1dr.view(np.uint8), w2blk.view(np.uint8), iota.view(np.uint8),
             dstof_c[c].view(np.uint8),
             b1h2.view(np.uint8), b2c.view(np.uint8)], axis=1)
        in_maps.append({
            "pack": pack_c[c],
            "persist": np.ascontiguousarray(per_c).view(f8),
        })
    flags = {"has_b1": bool(np.any(np.asarray(b1))),
             "has_b2": bool(np.any(np.asarray(b2)))}
    return T, in_maps, node_win, node_slot, flags


# ----------------------------------------------------------------------------
# Device program
# ----------------------------------------------------------------------------

def _patch_act_tables(table):
    """Force the activation-table chooser onto a single table so the ACT
    engine never reloads tables mid-kernel (1283ns per reload), and (for
    'softplus_and_others') expose the Softplus entry that act_info.json
    hides behind the opaque 'act2' slot name. Keys/order preserved so
    act_func_set_id stays valid."""
    import functools
    import concourse.bacc as bacc
    import concourse.hw_specs as hw_specs
    import concourse.mybir as mybir
    if getattr(bacc, "_act_tables_patched", None) == table:
        return
    real = hw_specs.get_activation_tables.__wrapped__ \
        if hasattr(hw_specs.get_activation_tables, "__wrapped__") \
        else hw_specs.get_activation_tables

    @functools.cache
    def only_shared(arch):
        tabs = dict(real(arch))
        out = {}
        for k, v in tabs.items():
            if k == table:
                v = set(v)
                if k == "softplus_and_others":
                    v.add(mybir.ActivationFunctionType.Softplus)
                out[k] = v
            else:
                out[k] = set()
        return out

    bacc.get_activation_tables = only_shared
    bacc._act_tables_patched = table


def _build(T, opt=None):
    import dataclasses as _dc
    import concourse.bass as bass
    import concourse.bacc as bacc
    import concourse.mybir as mybir
    import concourse.tile as tile

    EW = T * 128
    TD = T * 128               # msg/gath cols per pair

    opt = dict(opt or {})
    # sp1=True (single-pass hardware Softplus LUT) produced wrong results on
    # hardware, so default to the proven Exp+Ln two-pass softplus.
    SP1 = opt.get("sp1", False)
    PFD = opt.get("pfd", 2)             # group prefetch depth
    IOB = opt.get("iob", PFD + 2)
    WKB = opt.get("wkb", 3)
    STN = opt.get("stn", 6)             # st tile bufs
    PS1B = opt.get("ps1b", 2)
    PS2B = opt.get("ps2b", 2)
    PSOB = opt.get("psob", 2)
    CPD_MOD = opt.get("cpd_mod", 2)     # pso copy on DVE every CPD_MOD sp.
    HWREPS = opt.get("hwreps", 1)       # timing-only hardware loop
    HAS_B1 = opt.get("has_b1", True)
    HAS_B2 = opt.get("has_b2", True)

    _patch_act_tables("softplus_and_others" if SP1
                      else "natural_log_exp_and_others")

    fp32 = mybir.dt.float32
    bf16 = mybir.dt.bfloat16
    f8 = mybir.dt.float8e4
    OUTW = 2 * WSZ
    NG = NPAIR // GRP
    RBW = GRP * 2 * EW                  # rbf8 bytes per group row
    PKW = RBW + GRP * TD * 2            # full pack row (fp8 elements)
    # persist pack byte offsets
    PO_W1, PO_W2 = 0, 256
    PO_IOTA = PO_W2 + 256
    PO_DST = PO_IOTA + WSZ * 4 * T * 2
    PO_B1 = PO_DST + WPC * T * 2
    PO_B2 = PO_B1 + 4
    PB = PO_B2 + 4

    nc = bacc.Bacc("TRN2", target_bir_lowering=False, debug=False)

    pack = nc.dram_tensor("pack", [NG * 128, PKW], f8, kind="ExternalInput")
    persist = nc.dram_tensor("persist", [128, PB], f8, kind="ExternalInput")
    outd = nc.dram_tensor("out", [NG * DIM, GRP * OUTW], bf16,
                          kind="ExternalOutput")

    SOFT = mybir.ActivationFunctionType.Softplus
    EXP = mybir.ActivationFunctionType.Exp
    LN = mybir.ActivationFunctionType.Ln
    CP = mybir.ActivationFunctionType.Copy
    MUL = mybir.AluOpType.mult
    ADD = mybir.AluOpType.add
    EQ = mybir.AluOpType.is_equal
    DROW = mybir.MatmulPerfMode.DoubleRow

    with tile.TileContext(nc) as tc:
        with (
            tc.tile_pool(name="persist", bufs=1) as pp,
            tc.tile_pool(name="io", bufs=IOB) as io,
            tc.tile_pool(name="wk", bufs=WKB) as wk,
            tc.tile_pool(name="stp", bufs=STN) as stp,
            tc.tile_pool(name="stgp", bufs=2) as stgp,
            tc.tile_pool(name="ps1", bufs=PS1B, space="PSUM") as ps1p,
            tc.tile_pool(name="ps2", bufs=PS2B, space="PSUM") as ps2p,
            tc.tile_pool(name="pso", bufs=PSOB, space="PSUM") as psop,
        ):
            ppt = pp.tile([128, PB], f8)
            nc.gpsimd.dma_start(ppt[:], persist[:])
            w1_sb = ppt[:, PO_W1:PO_W1 + 256]                      # f8
            w2_sb = ppt[:, PO_W2:PO_W2 + 256].bitcast(bf16)        # [128,128]
            iota_sb = ppt[:, PO_IOTA:PO_DST].bitcast(bf16)
            dstof_sb = ppt[:, PO_DST:PO_B1].bitcast(bf16)
            b1h2_sb = ppt[:, PO_B1:PO_B1 + 4].bitcast(fp32)
            b2c_sb = ppt[0:DIM, PO_B2:PO_B2 + 4].bitcast(fp32)

            def load_group(g):
                pk4 = io.tile([128, PKW], f8, tag="pk", name="pk4")
                nc.gpsimd.dma_start(pk4[:], pack[g * 128:(g + 1) * 128, :])
                return pk4

            def body():
              # Software-pipelined over SUPERPAIRS (2 pairs = 4 windows per
              # iteration), 3 stages deep:
              #   A(i):  MM1 + one softplus + paired S_T builds (+ prefetch)
              #   B1(i-1): MM2 + gather-multiply (per pair)
              #   B2(i-2): scatter matmuls + one staging copy (+ group DMA)
              # so every cross-engine input is a full iteration old when
              # consumed and no engine's in-order stream closes a long
              # dependency cycle.
              NSUP = NPAIR // 2
              SPG = GRP // 2            # superpairs per DMA group
              grp_tiles = {g: load_group(g) for g in range(min(PFD, NG))}
              stg4 = None
              sb1 = sb2 = None
              for i in range(NSUP + 2):
                if i < NSUP:
                    g, j2 = divmod(i, SPG)
                    if j2 == 0:
                        if g + PFD < NG:
                            grp_tiles[g + PFD] = load_group(g + PFD)
                        pk4 = grp_tiles[g]

                    ps1 = ps1p.tile([128, 2 * EW], fp32, tag="mm1")
                    # One block-diag DoubleRow matmul per pair: window A's
                    # rbf on partitions 0:64, B on 64:128 (K=128 packed as
                    # 64 partitions x 2 fp8); the [[W1,0],[0,W1]] stationary
                    # routes A to out partitions 0:64 and B to 64:128 while
                    # using the full virtual 128x256 array.
                    for a in range(2):
                        jr = (2 * j2 + a) * 2 * EW
                        nc.tensor.matmul(
                            ps1[:, a * EW:(a + 1) * EW],
                            w1_sb.rearrange("p (s m) -> p s m", s=2),
                            pk4[:, jr:jr + 2 * EW].rearrange(
                                "p (s e) -> p s e", s=2),
                            start=True, stop=True, perf_mode=DROW)

                    # h1 = softplus(0.5*x + 0.5*b1); the reference's 2x
                    # output scale (beta=0.5) is folded into w2blk.  One
                    # ACT pass over both pairs (ps1 spans 2 PSUM banks).
                    h1 = wk.tile([128, 2 * EW], bf16, tag="h1")
                    if SP1:
                        nc.scalar.activation(
                            h1[:], ps1[:], SOFT,
                            bias=b1h2_sb if HAS_B1 else 0.0, scale=0.5)
                    else:
                        ex = wk.tile([128, 2 * EW], fp32, tag="ex")
                        nc.scalar.activation(
                            ex[:], ps1[:], EXP,
                            bias=b1h2_sb if HAS_B1 else 0.0, scale=0.5)
                        nc.scalar.activation(h1[:], ex[:], LN, bias=1.0)

                    # S_T2[p, n, (2a+sub)*T+t] = (dst_slot == n); ONE op
                    # covers all 4 windows of the superpair (fully packed
                    # last dims -> DVE 2x perf mode, and a single
                    # instruction's worth of semaphore traffic).
                    st = stp.tile([128, WSZ * 4 * T], bf16, tag="st")
                    _dv = dstof_sb[:, i * 4 * T:(i + 1) * 4 * T]
                    nc.vector.tensor_tensor(
                        out=st[:].rearrange("p (n u) -> p n u", u=4 * T),
                        in0=_dc.replace(
                            _dv, ap=[_dv.ap[0], [0, WSZ], [1, 4 * T]]),
                        in1=iota_sb.rearrange(
                            "p (n u) -> p n u", u=4 * T),
                        op=EQ)
                    sts = st

                # ---- B1: MM2 + gather-multiply for superpair i-1 ----
                nb1 = None
                if sb1 is not None:
                    pi, pj2, ph1, ppk4, psts = sb1
                    msgs = []
                    for a in range(2):
                        jt = (2 * pj2 + a) * TD
                        # pair-merged MM2: block-diag w2 -> (t, dA|dB)
                        ps2 = ps2p.tile([128, TD], fp32, tag="mm2")
                        for t in range(T):
                            nc.tensor.matmul(
                                ps2[:, t * 128:(t + 1) * 128],
                                ph1[:, a * EW + t * 128:
                                    a * EW + (t + 1) * 128], w2_sb,
                                start=True, stop=True)
                        msg = wk.tile([128, TD], bf16, tag="msg")
                        nc.vector.tensor_tensor(
                            out=msg[:], in0=ps2[:],
                            in1=ppk4[:, RBW + 2 * jt:
                                     RBW + 2 * (jt + TD)].bitcast(bf16),
                            op=MUL)
                        msgs.append(msg)
                    nb1 = (pi, pj2, ppk4, psts, msgs)

                # ---- B2: scatter matmuls + staging + group out DMA, i-2 ----
                if sb2 is not None:
                    pi, pj2, ppk4, psts, msgs = sb2
                    if pj2 == 0:
                        stg4 = stgp.tile([DIM, GRP * OUTW], bf16, tag="stg",
                                         name="stg4")

                    B2W = 2 if HAS_B2 else 1
                    pso = psop.tile([DIM, B2W * 2 * OUTW], fp32, tag="out")
                    _st = psts[:]
                    for a in range(2):
                        jt = (2 * pj2 + a) * TD
                        for sub in range(2):
                            # NOTE: accumulation groups must not interleave
                            # within one PSUM bank (HW corrupts the first).
                            st_ts = [
                                _dc.replace(
                                    _st,
                                    offset=_st.offset + (2 * a + sub) * T + t,
                                    ap=[_st.ap[0], [4 * T, WSZ]])
                                for t in range(T)]
                            col = (2 * a + sub) * WSZ
                            for t in range(T):
                                nc.tensor.matmul(
                                    pso[:, col:col + WSZ],
                                    msgs[a][:, t * 128 + sub * 64:
                                            t * 128 + sub * 64 + 64],
                                    st_ts[t], start=(t == 0),
                                    stop=(t == T - 1))
                            if HAS_B2:
                                for t in range(T):
                                    gt = ppk4[:, RBW + 2 * (jt + t * 128 + sub * 64):
                                              RBW + 2 * (jt + t * 128 + sub * 64 + 64)
                                              ].bitcast(bf16)
                                    nc.tensor.matmul(
                                        pso[:, 2 * OUTW + col:
                                            2 * OUTW + col + WSZ],
                                        gt,
                                        st_ts[t], start=(t == 0),
                                        stop=(t == T - 1))

                    jo = pj2 * 2 * OUTW
                    if HAS_B2:
                        # stg = pso_g * b2 + pso_msg (ACT scale + DVE add)
                        gb2 = wk.tile([DIM, 2 * OUTW], fp32, tag="gb2")
                        nc.scalar.activation(gb2[:],
                                             pso[:, 2 * OUTW:4 * OUTW],
                                             CP, bias=0.0, scale=b2c_sb)
                        nc.vector.tensor_tensor(
                            out=stg4[:, jo:jo + 2 * OUTW],
                            in0=pso[:, 0:2 * OUTW], in1=gb2[:], op=ADD)
                    elif pi % CPD_MOD == CPD_MOD - 1:
                        nc.vector.tensor_copy(stg4[:, jo:jo + 2 * OUTW],
                                              pso[:, 0:2 * OUTW])
                    else:
                        nc.scalar.activation(stg4[:, jo:jo + 2 * OUTW],
                                             pso[:, 0:2 * OUTW], CP)

                    # ONE output DMA per group, issued from the GpSimd
                    # (SWDGE) queue once the group's last staging copy is in.
                    pg = pi // SPG
                    if pj2 == SPG - 1:
                        nc.gpsimd.dma_start(
                            outd[pg * DIM:(pg + 1) * DIM, :], stg4[:])
                        grp_tiles.pop(pg, None)

                sb2 = nb1
                sb1 = (i, j2, h1, pk4, sts) if i < NSUP else None

            if HWREPS > 1:
                # two body copies per hardware-loop iteration: halves the
                # per-iteration all-engine barrier + pipeline drain/refill
                # overhead in the measured steady state
                KB = 2 if HWREPS % 2 == 0 else 1
                with tc.For_i(0, HWREPS // KB):
                    for _ in range(KB):
                        body()
            else:
                body()

    nc.compile()
    return nc


_CACHE = {}


def _get_nc(T, opt=None):
    key = (T, tuple(sorted((opt or {}).items())))
    if key not in _CACHE:
        _CACHE[key] = _build(T, opt)
    return _CACHE[key]


# ----------------------------------------------------------------------------
# Entry point
# ----------------------------------------------------------------------------

def kernel(rbf, new_node, src, dst, edge_mask, W1, b1, W2, b2):
    T, in_maps, node_win, node_slot, flags = _prepare(
        rbf, new_node, src, dst, edge_mask, W1, b1, W2, b2)
    nc = _get_nc(T, {"has_b1": flags["has_b1"], "has_b2": flags["has_b2"]})

    if os.environ.get("CFCONV_SIM"):
        outs = [_emulate_core(in_maps[c], T) for c in range(CORES)]
    else:
        from concourse.bass_utils import run_bass_kernel_spmd
        res = run_bass_kernel_spmd(nc, in_maps, core_ids=list(range(CORES)))
        outs = [r["out"] for r in res.results]

    # outs[c]: [NG*64, GRP*224] bf16; group row (g, d), col (j, ab, WSZ)
    arr = np.stack([np.asarray(o) for o in outs], 0).astype(np.float32)
    full = arr.reshape(CORES, NPAIR // GRP, DIM, GRP, 2, WSZ)
    full = full.transpose(2, 0, 1, 3, 4, 5).reshape(DIM, W_TOTAL * WSZ)
    col = node_win[:N_NODES] * WSZ + node_slot[:N_NODES]
    return np.ascontiguousarray(full[:, col].T.astype(np.float32))


def _emulate_core(in_map, T):
    """Numpy emulation of the device program for one core (debug only)."""
    import ml_dtypes
    bf = ml_dtypes.bfloat16
    f32 = np.float32
    EW = T * 128
    TD = T * 128
    NG = NPAIR // GRP
    RBW = GRP * 2 * EW

    packb = np.asarray(in_map["pack"]).view(np.uint8)      # [NG*128, PKW]
    rbf8 = packb[:, 0:RBW].view(ml_dtypes.float8_e4m3fn).astype(f32)
    gath = packb[:, RBW:].view(bf).astype(f32)             # [NG*128, GRP*TD]
    perb = np.asarray(in_map["persist"]).view(np.uint8)    # [128, PB]
    PO_W2 = 256
    PO_IOTA = PO_W2 + 256
    PO_DST = PO_IOTA + WSZ * 4 * T * 2
    PO_B1 = PO_DST + WPC * T * 2
    PO_B2 = PO_B1 + 4
    w1dr = perb[:, 0:256].view(ml_dtypes.float8_e4m3fn).astype(f32)
    w2blk = perb[:, PO_W2:PO_IOTA].view(bf).astype(f32)    # [128, 128]
    dstof = perb[:, PO_DST:PO_B1].view(bf).astype(f32)     # [128, WPC*T]
    b1h2 = perb[:, PO_B1:PO_B1 + 4].view(f32)              # [128, 1]
    b2c = perb[0:64, PO_B2:PO_B2 + 4].view(f32)            # [64, 1]

    # regroup [NG*128, GRP*X] -> per-pair blocks [NPAIR*128, X]
    rbf8 = rbf8.reshape(NG, 128, GRP, 2 * EW)
    rbf8 = rbf8.transpose(0, 2, 1, 3).reshape(NPAIR * 128, 2 * EW)
    gath = gath.reshape(NG, 128, GRP, TD)
    gath = gath.transpose(0, 2, 1, 3).reshape(NPAIR * 128, TD)

    w1blk = w1dr.reshape(128, 2, 128)
    outp = np.zeros((NPAIR * DIM, 2 * WSZ), f32)
    for q in range(NPAIR):
        blk = rbf8[q * 128:(q + 1) * 128].reshape(128, 2, EW)
        gat = gath[q * 128:(q + 1) * 128]          # [128, TD]
        # block-diag DoubleRow MM1: ps1[M, e] = sum_{P,s} w1blk[P,s,M] *
        # rbf_pair[P, s, e]
        ps1 = np.einsum("psm,pse->me", w1blk, blk).astype(f32)
        h1 = np.log1p(np.exp(0.5 * ps1 + b1h2)).astype(bf).astype(f32)
        ps2 = np.zeros((128, TD), f32)
        for t in range(T):
            ps2[:, t * 128:(t + 1) * 128] = \
                h1[:, t * 128:(t + 1) * 128].T @ w2blk
        msg = (ps2 * gat).astype(bf).astype(f32)
        pso = np.zeros((DIM, 2 * WSZ), f32)
        psg = np.zeros((DIM, 2 * WSZ), f32)
        for sub in range(2):
            w = 2 * q + sub
            dof = dstof[:, w * T:(w + 1) * T]
            stt = (dof[:, :, None] == np.arange(WSZ)[None, None, :])
            for t in range(T):
                s = stt[:, t].astype(f32)          # [128, WSZ]
                m = msg[:, t * 128 + sub * 64:t * 128 + sub * 64 + 64]
                g = gat[:, t * 128 + sub * 64:t * 128 + sub * 64 + 64]
                pso[:, sub * WSZ:(sub + 1) * WSZ] += m.T @ s
                psg[:, sub * WSZ:(sub + 1) * WSZ] += g.T @ s
        outp[q * DIM:(q + 1) * DIM] = pso + b2c * psg
    # repack per-pair [NPAIR*64, 224] -> grouped [NG*64, GRP*224]
    outp = outp.reshape(NG, GRP, DIM, 2 * WSZ).transpose(0, 2, 1, 3)
    return outp.reshape(NG * DIM, GRP * 2 * WSZ).astype(bf)
